# revision 55
# baseline (speedup 1.0000x reference)
"""ConcatCritic MLP over the B^2 pair grid, Trainium2 Bass/Tile kernel.

out[i, j] = softplus(f(x[i], y[j])) where f is a 4-layer MLP on
concat(x, y). Reference pair grid: pairs[a*n+b] = concat(x[b], y[a]),
scores.reshape(n,n).T -> out.

Key factorization: layer 1 is linear in the concat, so
  h1[a,b] = relu(x[b] @ W1top + y[a] @ W1bot + b1)
with W1top = W1[:128], W1bot = W1[128:]. The [B^2, 256] layer-1 matmul
collapses into two tiny matmuls plus a per-partition broadcast add.

Layout: activations kept transposed [features, batch] so every layer's
matmul (weights stationary as lhsT [K, M]) writes the next layer's rhs
directly: out[m=feat, n=j] = sum_k W[k, m] * hT[k, j]. fp32r keeps the
PE at 1 cycle/row (fp8 DoubleRow would halve PE time but fails the
accuracy budget: e4m3 acts+weights measured 4e-2 rel err vs 2e-2 gate).

L4 (M=1) runs column-tiled in bf16: 4 chunks' score rows land on PE
col-strips 32c of ONE psum tile (tile_position=(0, 32c)); the 8 strip
matmuls of a group stream concurrently, costing ~2 matmul slots instead
of 8. Each group's psum is drained by one ACT Exp over all 128
partitions (rows 32c real, rest junk) and a single partition-strided
DMA gather of rows {0,32,64,96}; softplus finishes as a batched
Ln(1+e) pair at the very end (a single end tail keeps the Ln table
switch out of the chunk loop — a hoisted switch costs two mid-loop
table loads and a PE stall).

Per chunk the pointwise epilogues are split DVE/ACT to balance busy
time (~80% each): DVE takes both L1 tensor_scalars + L2epi mc1 (+L3epi
mc0 on even chunks); ACT takes the rest. Pipeline skew: L1 two steps
ahead, L3 two steps behind L2, L4 batches between L2MM and L3MM.
PSUM: l2 2 tags x bufs2 + l3 2 x bufs2, with the L4 group tile sharing
the l2_0 rotation (8 banks exactly).

Sharding: core c owns y rows [c*64, (c+1)*64); computes block
V_c[il, j] = f(x[j], y[c*64+il]) of shape [64, 512]. Host gathers
V = concat(V_c) and returns V.T.

Measured on trn2: ~159 us HW exec (baseline 182 us), rel err 1.6e-3.

Self-contained: hardcodes shapes; imports concourse from the system repo.
"""

import os
import sys

import numpy as np


def _import_concourse():
    try:
        import concourse  # noqa: F401
        return
    except ImportError:
        pass
    for p in ("/opt/trn_rl_repo", "/root/.axon_site/_ro/trn_rl_repo"):
        if os.path.isdir(p) and p not in sys.path:
            sys.path.insert(0, p)
    import concourse  # noqa: F401


_import_concourse()

import concourse.bacc as bacc  # noqa: E402
import concourse.tile as tile  # noqa: E402
from concourse import mybir  # noqa: E402
from concourse.bass_utils import run_bass_kernel_spmd  # noqa: E402

B = 512          # batch (pair-grid side)
D = 128          # input dim per tensor
H = 256          # hidden dim
NCORES = 8
RB = B // NCORES  # 64 y-rows per core
GRP = 4           # chunks per L4 column-tiled group
NGRP = RB // GRP
F32 = mybir.dt.float32

# float32r: fp32 bits in memory, single-pass reduced-precision multiply on
# the PE at 1 cycle/row (vs 4 for full fp32).
MM_DT = mybir.dt.float32r
# L4 runs in bf16: walrus rejects tile_position'd matmuls with 4-byte
# self-loading weights, and bf16 h3/w4 keeps the col-tiled path on the
# documented recipe (16-bit weights, M=32 strips).
BF16 = mybir.dt.bfloat16


def _emit(tc, nc, d, out_d):
    AF = mybir.ActivationFunctionType
    OP = mybir.AluOpType
    from contextlib import ExitStack

    with ExitStack() as ctx:
        const = ctx.enter_context(tc.tile_pool(name="const", bufs=1))
        hpool = ctx.enter_context(tc.tile_pool(name="h", bufs=2))
        psum = ctx.enter_context(tc.tile_pool(name="psum", bufs=1, space="PSUM"))

        # Warm the ACT function table before anything else: the first table
        # load overlaps the weight DMAs instead of stalling the xa epilogue.
        warm = const.tile([1, 1], F32, tag="warm", name="warm")
        nc.scalar.activation(warm[:], warm[:], AF.Identity)

        def load(name, shape, src_ap=None, dt=F32, eng=None):
            t = const.tile(list(shape), dt, tag=name, name=name + "_s")
            src = src_ap if src_ap is not None else d[name][:]
            if src.dtype != dt:
                src = src.bitcast(dt)
            (eng or nc.sync).dma_start(out=t[:], in_=src)
            return t

        # Weight tensors in first-use order on the SP DMA ring (yb matmuls
        # run first, then xa). Tiny bias tiles go through the Activation
        # HWDGE ring so they land immediately instead of queueing behind
        # ~1MB of weight traffic.
        yT = load("yT", (D, RB), dt=MM_DT)
        w1b = load("W1b", (D, H), dt=MM_DT)
        xT = load("xT", (D, B), dt=MM_DT)
        w1t = load("W1t", (D, H), dt=MM_DT)
        b1c = [load(f"b1_{k}", (128, 1), d["b1"][k * 128:(k + 1) * 128, :], eng=nc.scalar) for k in range(2)]
        b2c = [load(f"b2_{k}", (128, 1), d["b2"][k * 128:(k + 1) * 128, :], eng=nc.scalar) for k in range(2)]
        b3c = [load(f"b3_{k}", (128, 1), d["b3"][k * 128:(k + 1) * 128, :], eng=nc.scalar) for k in range(2)]
        b4f = load("b4f", (128, 1), eng=nc.scalar)
        w2 = [load(f"W2_{k}", (128, H), d["W2"][k * 128:(k + 1) * 128, :], MM_DT) for k in range(2)]
        w3 = [load(f"W3_{k}", (128, H), d["W3"][k * 128:(k + 1) * 128, :], MM_DT) for k in range(2)]
        w4r = [load(f"W4r_{k}", (128, 32), d[f"W4r_{k}"][:], BF16) for k in range(2)]

        # ---- preamble: xa[oc] = (x @ W1top)^T + b1 (bias folded here),
        #                yb[oc] = (y_slice @ W1bot)^T
        # Preamble psum tiles borrow main-loop tags (no extra banks).
        xa = []
        yb = []
        for oc in range(2):
            ms = slice(oc * 128, (oc + 1) * 128)
            pyb = psum.tile([128, RB], F32, tag=f"l3_{oc}", name=f"pyb{oc}", bufs=2)
            nc.tensor.matmul(pyb[:], lhsT=w1b[:, ms], rhs=yT[:], start=True, stop=True)
            ybt = const.tile([128, RB], F32, tag=f"yb{oc}", name=f"yb{oc}")
            nc.vector.tensor_copy(ybt[:], pyb[:])
            yb.append(ybt)
        for oc in range(2):
            ms = slice(oc * 128, (oc + 1) * 128)
            pxa = psum.tile([128, B], F32, tag=f"l2_{oc}", name=f"pxa{oc}", bufs=2)
            nc.tensor.matmul(pxa[:], lhsT=w1t[:, ms], rhs=xT[:], start=True, stop=True)
            xat = const.tile([128, B], MM_DT, tag=f"xa{oc}", name=f"xa{oc}")
            nc.scalar.activation(xat[:], pxa[:], AF.Identity, bias=b1c[oc][:, 0:1])
            xa.append(xat)

        # Raw score+b4 rows for all chunks, gathered via DMA from the
        # per-group stage tiles; one batched Softplus finishes at the end.
        escore = const.tile([RB, B], F32, tag="escore", name="escore")

        h1s, h2s, h3s = {}, {}, {}

        def emit_l1(i):
            for oc in range(2):
                t = hpool.tile([128, B], MM_DT, tag=f"h1_{oc}", name=f"h1_{oc}_{i}", bufs=4)
                nc.vector.tensor_scalar(
                    t[:], xa[oc][:], yb[oc][:, i:i + 1], 0.0, OP.add, OP.max
                )
                h1s[i, oc] = t

        def emit_l2mm(i):
            for mc in range(2):
                ms = slice(mc * 128, (mc + 1) * 128)
                p = psum.tile([128, B], F32, tag=f"l2_{mc}", name=f"p2_{mc}_{i}", bufs=2)
                for kc in range(2):
                    nc.tensor.matmul(
                        p[:], lhsT=w2[kc][:, ms], rhs=h1s[i, kc][:],
                        start=(kc == 0), stop=(kc == 1),
                    )
                h2s[i, mc] = p

        def emit_l2epi(i):
            # mc=0 on ACT, mc=1 on DVE (engine balance).
            for mc in range(2):
                p = h2s[i, mc]
                t = hpool.tile([128, B], MM_DT, tag=f"h2_{mc}", name=f"h2_{mc}_{i}", bufs=4)
                if mc == 0:
                    nc.scalar.activation(t[:], p[:], AF.Relu, bias=b2c[mc][:, 0:1])
                else:
                    nc.vector.tensor_scalar(t[:], p[:], b2c[mc][:, 0:1], 0.0, OP.add, OP.max)
                h2s[i, mc] = t

        # DVE carries L1 x2 + L2epi mc=1 every chunk; the fourth flexible op
        # (one L3epi half) alternates DVE/ACT so busy time balances at
        # ~3.5/2.5 ops (DVE 543 ns/op vs ACT 659 ns/op + per-group Exp).

        def emit_l3mm(i):
            for mc in range(2):
                ms = slice(mc * 128, (mc + 1) * 128)
                p = psum.tile([128, B], F32, tag=f"l3_{mc}", name=f"p3_{mc}_{i}", bufs=2)
                for kc in range(2):
                    nc.tensor.matmul(
                        p[:], lhsT=w3[kc][:, ms], rhs=h2s[i, kc][:],
                        start=(kc == 0), stop=(kc == 1),
                    )
                h3s[i, mc] = p
            del h1s[i, 0], h1s[i, 1]

        def emit_l3epi(i):
            for mc in range(2):
                p = h3s[i, mc]
                t = hpool.tile([128, B], BF16, tag=f"h3_{mc}", name=f"h3_{mc}_{i}", bufs=6)
                if mc == 0 and i % 2 == 0:
                    nc.vector.tensor_scalar(t[:], p[:], b3c[mc][:, 0:1], 0.0, OP.add, OP.max)
                else:
                    nc.scalar.activation(t[:], p[:], AF.Relu, bias=b3c[mc][:, 0:1])
                h3s[i, mc] = t
            del h2s[i, 0], h2s[i, 1]

        l4ps = {}

        def emit_l4mms(g):
            # 4 chunks' L4 on one psum tile, column strips 32c. kc-major
            # order: the 4 kc=0 strip matmuls stream concurrently (col
            # tiling), each strip's kc=1 follows as soon as its kc=0 drains,
            # overlapping the other strips. One contiguous batch costs
            # ~2 matmul slots + a single drain stall for the whole group.
            # Shares the l2_0 bank rotation (PSUM is 8 banks: 2x2 l2 + 2x2 l3
            # leaves none spare); the drain empties it well before l2 cycles
            # back.
            l4ps[g] = ps = psum.tile([128, B], F32, tag="l2_0", name=f"p4_{g}", bufs=2)
            for kc in range(2):
                for c in range(GRP):
                    i = g * GRP + c
                    nc.tensor.matmul(
                        ps[32 * c:32 * c + 32, :], lhsT=w4r[kc][:],
                        rhs=h3s[i, kc][:],
                        start=(kc == 0), stop=(kc == 1),
                        tile_position=(0, 32 * c),
                        skip_group_check=True,
                    )

        def emit_l4drain(g):
            ps = l4ps.pop(g)
            for c in range(GRP):
                i = g * GRP + c
                del h3s[i, 0], h3s[i, 1]
            # One Exp over all 128 partitions (rows 32c are real, rest junk),
            # then gather the 4 real rows into escore.
            st = hpool.tile([128, B], F32, tag="stage", name=f"st_{g}")
            nc.scalar.activation(st[:], ps[:], AF.Exp, bias=b4f[:, 0:1])
            nc.sync.dma_start(
                out=escore[g * GRP:(g + 1) * GRP, :],
                in_=st[0:128:32, :],
            )

        # ---- batched tail: ln(1 + e) over all 64 rows in one ACT op.
        # A single op depending on the full escore keeps the tile scheduler
        # from hoisting it (and its Ln table switch) into the chunk loop —
        # a hoisted switch costs two mid-loop table loads and a PE stall.
        fin = const.tile([RB, B], F32, tag="fin", name="fin")

        def emit_tail():
            # Two halves so the first half's output DMA overlaps the second
            # Ln; both use the same table set (one load).
            HALF = RB // 2
            nc.scalar.activation(fin[0:HALF, :], escore[0:HALF, :], AF.Ln, bias=1.0)
            nc.sync.dma_start(out=out_d[0:HALF, :], in_=fin[0:HALF, :])
            nc.scalar.activation(fin[HALF:RB, :], escore[HALF:RB, :], AF.Ln, bias=1.0)
            nc.sync.dma_start(out=out_d[HALF:RB, :], in_=fin[HALF:RB, :])

        # Software pipeline: step t runs L2MM(t) | L4 batch | L3MM(t-2).
        # The two-step L2->L3 skew gives the h2 epilogues a full step of
        # slack before L3 consumes them.
        emit_l1(0)
        emit_l1(1)
        for t in range(RB + 3):
            g = (t - 3) // GRP if t >= 3 and (t - 3) % GRP == GRP - 1 else None
            if t + 2 < RB:
                emit_l1(t + 2)
            if t < RB:
                emit_l2mm(t)
                emit_l2epi(t)
            if g is not None:
                emit_l4mms(g)
            if t >= 2 and t - 2 < RB:
                emit_l3mm(t - 2)
                emit_l3epi(t - 2)
            if g is not None:
                emit_l4drain(g)

        emit_tail()


def _build_program():
    nc = bacc.Bacc("TRN2", target_bir_lowering=False, debug=False, enable_asserts=False)
    d = {}
    for name, shape in [
        ("xT", (D, B)), ("yT", (D, RB)),
        ("W1t", (D, H)), ("W1b", (D, H)),
        ("W2", (H, H)), ("W3", (H, H)),
        ("b1", (H, 1)), ("b2", (H, 1)), ("b3", (H, 1)), ("b4f", (128, 1)),
    ]:
        d[name] = nc.dram_tensor(name, list(shape), F32, kind="ExternalInput").ap()
    for k in range(2):
        d[f"W4r_{k}"] = nc.dram_tensor(
            f"W4r_{k}", [128, 32], BF16, kind="ExternalInput"
        ).ap()
    out_d = nc.dram_tensor("out", [RB, B], F32, kind="ExternalOutput").ap()
    with tile.TileContext(nc) as tc:
        _emit(tc, nc, d, out_d)
    nc.compile()
    return nc


_PROGRAM = None


def _get_program():
    global _PROGRAM
    if _PROGRAM is None:
        _PROGRAM = _build_program()
    return _PROGRAM


def _make_in_maps(x, y, W1, b1, W2, b2, W3, b3, W4, b4):
    import ml_dtypes

    f = np.float32
    xT = np.ascontiguousarray(x.T, dtype=f)
    w4c = np.asarray(W4, dtype=f).reshape(H, 1)
    shared = {
        "xT": xT,
        "W1t": np.ascontiguousarray(W1[:D], dtype=f),
        "W1b": np.ascontiguousarray(W1[D:], dtype=f),
        "W2": np.ascontiguousarray(W2, dtype=f),
        "W3": np.ascontiguousarray(W3, dtype=f),
        "W4r_0": np.ascontiguousarray(
            np.tile(w4c[:128], (1, 32)).astype(ml_dtypes.bfloat16)),
        "W4r_1": np.ascontiguousarray(
            np.tile(w4c[128:], (1, 32)).astype(ml_dtypes.bfloat16)),
        "b1": np.ascontiguousarray(b1.reshape(H, 1), dtype=f),
        "b2": np.ascontiguousarray(b2.reshape(H, 1), dtype=f),
        "b3": np.ascontiguousarray(b3.reshape(H, 1), dtype=f),
        "b4f": np.full((128, 1), np.asarray(b4, dtype=f).reshape(-1)[0], dtype=f),
    }
    in_maps = []
    for c in range(NCORES):
        m = dict(shared)
        m["yT"] = np.ascontiguousarray(y[c * RB:(c + 1) * RB].T, dtype=f)
        in_maps.append(m)
    return in_maps


def _run(inputs, trace=False, trace_cores=None):
    nc = _get_program()
    in_maps = _make_in_maps(**inputs)
    res = run_bass_kernel_spmd(
        nc, in_maps, list(range(NCORES)), trace=trace, trace_cores=trace_cores,
    )
    V = np.concatenate([res.results[c]["out"] for c in range(NCORES)], axis=0)
    out = np.ascontiguousarray(V.T, dtype=np.float32)
    return out, res


def kernel(**inputs):
    out, _ = _run(inputs, trace=False)
    return out
